# revision 7
# baseline (speedup 1.0000x reference)
"""DenoiserNetwork Trainium2 kernel: 4-layer folded RepVGG conv stack + a-trous filtering.

Sharding: 8 cores = batch(4) x H-halves(2). Full inputs in, full output out.
"""
import sys, types
import numpy as np

IN_C, MID_C, NUM_LAYERS, KLEV = 3, 16, 4, 4
H = W = 1024
RS = 512          # rows per shard
N_CORES = 8
NBX = 67          # xconv blocks (rows r0-8-2pad.., 8 rows each)
NB = [66, 66, 65, 64]  # out-block counts for L0..L3
RING0 = 4
RING = 4


def _install_ntff_hook():
    try:
        import antenv
        if "antenv.axon_hooks" in sys.modules:
            return
        hooks_mod = types.ModuleType("antenv.axon_hooks")
        _h = [None]
        hooks_mod.set_axon_ntff_profile_hook = lambda h: _h.__setitem__(0, h)
        hooks_mod.get_axon_ntff_profile_hook = lambda: _h[0]
        sys.modules["antenv.axon_hooks"] = hooks_mod
        antenv.axon_hooks = hooks_mod
        from trn_agent_boot.trn_boot import _ntff_profile_via_ctypes
        hooks_mod.set_axon_ntff_profile_hook(
            _ntff_profile_via_ctypes('/opt/axon/libaxon_pjrt.so'))
    except Exception:
        pass


# ---------------------------------------------------------------- host prep --

def _fold_params(params):
    """RepVGG fold: each layer -> (Weff [co,ci,5,5], beff [co])."""
    folded = []
    for p in params:
        w5 = np.asarray(p['w5'], np.float32)
        w3 = np.asarray(p['w3'], np.float32)
        w1 = np.asarray(p['w1'], np.float32)
        co, ci = w5.shape[0], w5.shape[1]
        w = w5.copy()
        w[:, :, 1:4, 1:4] += w3
        w[:, :, 2:3, 2:3] += w1
        if co == ci:
            for c in range(co):
                w[c, c, 2, 2] += 1.0
        b = (np.asarray(p['b5']) + np.asarray(p['b3']) + np.asarray(p['b1'])).astype(np.float32)
        folded.append((w, b))
    return folded


def _pack_lhsT(weff, n_i, co_n, ci_n, packed_kw):
    """Build (g1, g2) lhsT arrays.

    g1[(j,(kw,)ci), (i*co_n+co)] = weff[co,ci,j-i,kw]   (0<=j-i<=4), j in 0..7
    g2[(j2,(kw,)ci), m]          = weff[co,ci,j2+8-i,kw], j2 in 0..3
    Without packed_kw returns per-kw lists.
    """
    co = co_n
    if packed_kw:
        g1 = np.zeros((8, 5, ci_n, n_i * co_n), np.float32)
        g2 = np.zeros((4, 5, ci_n, n_i * co_n), np.float32)
        for j in range(8):
            for kw in range(5):
                for ci in range(ci_n):
                    for i in range(n_i):
                        kh = j - i
                        if 0 <= kh <= 4:
                            g1[j, kw, ci, i * co_n:(i + 1) * co_n] = weff[:, ci, kh, kw]
                        kh2 = (j + 8 - i) if j < 4 else -1
                        if j < 4 and 0 <= kh2 <= 4:
                            g2[j, kw, ci, i * co_n:(i + 1) * co_n] = weff[:, ci, kh2, kw]
        return (g1.reshape(40, ci_n, n_i * co_n)[:, :, :].reshape(8 * 5 * ci_n, -1),
                g2[:4].reshape(4 * 5 * ci_n, -1))
    else:
        g1s, g2s = [], []
        for kw in range(5):
            g1 = np.zeros((8, ci_n, n_i * co_n), np.float32)
            g2 = np.zeros((4, ci_n, n_i * co_n), np.float32)
            for j in range(8):
                for ci in range(ci_n):
                    for i in range(n_i):
                        kh = j - i
                        if 0 <= kh <= 4:
                            g1[j, ci, i * co_n:(i + 1) * co_n] = weff[:, ci, kh, kw]
            for j2 in range(4):
                for ci in range(ci_n):
                    for i in range(n_i):
                        kh = j2 + 8 - i
                        if 0 <= kh <= 4:
                            g2[j2, ci, i * co_n:(i + 1) * co_n] = weff[:, ci, kh, kw]
            g1s.append(g1.reshape(8 * ci_n, -1))
            g2s.append(g2.reshape(4 * ci_n, -1))
        return np.stack(g1s, axis=1), np.stack(g2s, axis=1)


def _host_inputs_for_core(core, imgs, folded):
    b, half = core // 2, core % 2
    r0 = half * RS
    x = imgs[b]  # [3, H, W]

    # xconv: [120, NBX, 1024] kw-packed zero-padded slab, base row r0-10
    xc = np.zeros((8, 5, IN_C, NBX, W), np.float32)
    base0 = r0 - 8
    for j in range(8):
        rr = base0 + 8 * np.arange(NBX) + j  # [NBX]
        valid_r = (rr >= 0) & (rr < H)
        for kw in range(5):
            csrc = np.arange(W) + kw - 2
            valid_c = (csrc >= 0) & (csrc < W)
            cc = np.clip(csrc, 0, W - 1)
            block = x[:, np.clip(rr, 0, H - 1)[:, None], cc[None, :]]  # [3, NBX, W]
            block = block * valid_r[None, :, None] * valid_c[None, None, :]
            xc[j, kw] = block
    xconv = xc.reshape(120, NBX, W)

    # xfilt: [3, 528, 1040] circular wrap, rows r0-8.., cols -8..
    rr = (np.arange(RS + 16) + r0 - 8) % H
    cc = (np.arange(W + 16) - 8) % W
    xfilt = x[:, rr[:, None], cc[None, :]].astype(np.float32)

    # masks for slab1..3 out-blocks: mask_l[p, t] row-valid indicator
    masks = np.zeros((3, 128, 68), np.float32)
    for l, base in enumerate((r0 - 6, r0 - 4, r0 - 2)):
        for t in range(68):
            for j in range(8):
                r = base + 8 * t + j
                masks[l, j * 16:(j + 1) * 16, t] = 1.0 if 0 <= r < H else 0.0
    return xconv, xfilt, masks, r0


def _host_weights(folded):
    w0g1, w0g2 = _pack_lhsT(folded[0][0], 8, MID_C, IN_C, True)
    w1g1, w1g2 = _pack_lhsT(folded[1][0], 8, MID_C, MID_C, False)
    w2g1, w2g2 = _pack_lhsT(folded[2][0], 8, MID_C, MID_C, False)
    w3g1, w3g2 = _pack_lhsT(folded[3][0], 8, 2 * KLEV, MID_C, False)
    biases = np.zeros((4, 128), np.float32)
    for l in range(4):
        co_n = 2 * KLEV if l == 3 else MID_C
        for i in range(8):
            biases[l, i * co_n:(i + 1) * co_n] = folded[l][1]
    return dict(w0g1=w0g1, w0g2=w0g2, w1g1=w1g1, w1g2=w1g2,
                w2g1=w2g1, w2g2=w2g2, w3g1=w3g1, w3g2=w3g2, biases=biases)


# ---------------------------------------------------------------- bass build --

def _build(nc):
    import concourse.bass as bass
    import concourse.tile as tile
    from concourse import mybir
    dt, AF, ALU = mybir.dt, mybir.ActivationFunctionType, mybir.AluOpType
    F32, F32R = dt.float32, dt.float32r

    xconv_d = nc.dram_tensor("xconv", [120, NBX, W], F32R, kind="ExternalInput")
    xfilt_d = nc.dram_tensor("xfilt", [IN_C, RS + 16, W + 16], F32, kind="ExternalInput")
    masks_d = nc.dram_tensor("masks", [3, 128, 68], F32, kind="ExternalInput")
    w_d = {}
    wshapes = dict(w0g1=[120, 128], w0g2=[60, 128],
                   w1g1=[128, 5, 128], w1g2=[64, 5, 128],
                   w2g1=[128, 5, 128], w2g2=[64, 5, 128],
                   w3g1=[128, 5, 64], w3g2=[64, 5, 64])
    for k, shp in wshapes.items():
        w_d[k] = nc.dram_tensor(k, shp, F32R, kind="ExternalInput")
    bias_d = nc.dram_tensor("biases", [4, 128], F32, kind="ExternalInput")
    zeros_d = nc.dram_tensor("zeros2", [128, 2], F32R, kind="ExternalInput")
    y_d = nc.dram_tensor("y", [IN_C, RS, W], F32, kind="ExternalOutput")
    guide_dram = nc.dram_tensor("guide_scratch", [2 * KLEV, RS, W], F32)  # internal

    with tile.TileContext(nc) as tc:
        import contextlib
        ctx = contextlib.ExitStack()
        with ctx:
            singles = ctx.enter_context(tc.tile_pool(name="singles", bufs=1))
            psum = ctx.enter_context(tc.tile_pool(name="psum", bufs=6, space="PSUM"))
            stage = ctx.enter_context(tc.tile_pool(name="stage", bufs=2))
            fp2 = ctx.enter_context(tc.tile_pool(name="fp2", bufs=2))
            fp1 = ctx.enter_context(tc.tile_pool(name="fp1", bufs=1))
            cpool = ctx.enter_context(tc.tile_pool(name="cpool", bufs=1))

            # weights/bias/mask loads
            wt = {}
            for k, shp in wshapes.items():
                wt[k] = singles.tile(shp, F32R, name=f'w_{k}')
                nc.sync.dma_start(wt[k][:], w_d[k][:])
            bias_t = singles.tile([128, 4], F32)
            nc.sync.dma_start(bias_t[:], bias_d[:].rearrange("l p -> p l"))
            mask_t = singles.tile([128, 3, 68], F32)
            nc.sync.dma_start(mask_t[:], masks_d[:].rearrange("l p t -> p l t"))

            # ring slabs
            slab0 = singles.tile([120, RING0, W], F32R, name='slab0')
            slab = [singles.tile([128, RING, W + 4], F32R, name=f'slab{i+1}') for i in range(3)]
            # zero borders of slab1..3 (cols 0,1 and W+2,W+3) via broadcast DMA
            zsrc = bass.AP(tensor=zeros_d.ap().tensor, offset=0,
                           ap=[[2, 128], [0, RING], [1, 2]])
            for s in slab:
                nc.sync.dma_start(s[:, :, 0:2], zsrc)
                nc.sync.dma_start(s[:, :, W + 2:W + 4], zsrc)

            def load_x_block(t):
                if t < NBX:
                    nc.sync.dma_start(slab0[:, t % RING0, :], xconv_d[:, t, :])

            def conv_block(l, t):
                """Compute layer l out-block t (both halves)."""
                if l == 0:
                    src, g1, g2, co_n = slab0, wt["w0g1"], wt["w0g2"], MID_C
                    r_in = RING0
                else:
                    src = slab[l - 1]
                    g1, g2 = wt[f"w{l}g1"], wt[f"w{l}g2"]
                    co_n = 2 * KLEV if l == 3 else MID_C
                    r_in = RING
                m = 8 * co_n
                for h in range(2):
                    acc = psum.tile([128, 512], F32, tag="psacc")
                    if l == 0:
                        nc.tensor.matmul(acc[:m], g1[:], slab0[:, t % RING0, h * 512:h * 512 + 512],
                                         start=True, stop=False)
                        nc.tensor.matmul(acc[:m], g2[:], slab0[:60, (t + 1) % RING0, h * 512:h * 512 + 512],
                                         start=False, stop=True)
                    else:
                        for kw in range(5):
                            nc.tensor.matmul(acc[:m], g1[:, kw, :],
                                             src[:, t % r_in, h * 512 + kw:h * 512 + kw + 512],
                                             start=(kw == 0), stop=False)
                        for kw in range(5):
                            nc.tensor.matmul(acc[:m], g2[:, kw, :],
                                             src[:64, (t + 1) % r_in, h * 512 + kw:h * 512 + kw + 512],
                                             start=False, stop=(kw == 4))
                    # evacuate with relu + bias
                    if l < 3:
                        dst = slab[l][:m, t % RING, 2 + h * 512: 2 + h * 512 + 512]
                        if (t + h) % 2 == 0:
                            nc.scalar.activation(dst, acc[:m], AF.Relu,
                                                 bias=bias_t[:m, l:l + 1], scale=1.0)
                        else:
                            nc.vector.tensor_scalar(dst, acc[:m],
                                                    scalar1=bias_t[:m, l:l + 1], scalar2=0.0,
                                                    op0=ALU.add, op1=ALU.max)
                        if t <= 1 or t >= NB[l] - 2:
                            nc.vector.tensor_scalar_mul(dst, dst, mask_t[:m, l, t:t + 1])
                    else:
                        st = stage.tile([64, 512], F32, tag="gstage")
                        nc.scalar.activation(st[:], acc[:64], AF.Relu,
                                             bias=bias_t[:64, 3:4], scale=1.0)
                        # staging partitions m = i*8+co ; scatter to guide_dram[co, r0loc+8t+i, h*512+c]
                        dst_ap = bass.AP(
                            tensor=guide_dram.ap().tensor, offset=8 * t * W + h * 512,
                            ap=[[W, 8], [RS * W, 8], [1, 512]])
                        nc.sync.dma_start(dst_ap, st[:])

            # filtering constants
            DIL = [1, 2, 4, 8]

            def filtering(band, h):
                ch0 = h * 512
                gh = fp2.tile([128, 8, 512], F32, tag="guideh", name="gh")
                src_ap = bass.AP(tensor=guide_dram.ap().tensor,
                                 offset=band * 128 * W + ch0,
                                 ap=[[W, 128], [RS * W, 8], [1, 512]])
                nc.sync.dma_start(gh[:], src_ap)
                # softmax over ch 0..3 (no max-subtract; relu'd inputs are small)
                e_sm = fp1.tile([128, 4, 512], F32, tag="esm", name="e_sm")
                for l in range(KLEV):
                    nc.scalar.activation(e_sm[:, l, :], gh[:, l, :], AF.Exp)
                ssum = cpool.tile([128, 512], F32, tag="ssum")
                nc.vector.tensor_add(ssum[:], e_sm[:, 0, :], e_sm[:, 1, :])
                t2 = cpool.tile([128, 512], F32, tag="t2", name="t2")
                nc.vector.tensor_add(t2[:], e_sm[:, 2, :], e_sm[:, 3, :])
                nc.vector.tensor_add(ssum[:], ssum[:], t2[:])
                rsm = cpool.tile([128, 512], F32, tag="rsm")
                nc.vector.reciprocal_approx_fast(rsm[:], ssum[:])

                # v0 slab [128, 3, 528]
                v0 = fp1.tile([128, IN_C, 528], F32, tag="v0", name="v0")
                src = bass.AP(tensor=xfilt_d.ap().tensor,
                              offset=(band * 128 + 8) * (W + 16) + ch0,
                              ap=[[W + 16, 128], [(RS + 16) * (W + 16), IN_C], [1, 528]])
                nc.sync.dma_start(v0[:], src)

                out = fp1.tile([128, IN_C, 512], F32, tag="outb", name="out")
                asum = cpool.tile([128, 512], F32, tag="asum")
                first = True
                for l in range(KLEV):
                    d = DIL[l]
                    # S = xfilt[+d] + xfilt[-d] via accumulating DMA
                    S = fp1.tile([128, IN_C, 528], F32, tag="S", name="S")
                    for sgn in (d, -d):
                        src = bass.AP(tensor=xfilt_d.ap().tensor,
                                      offset=(band * 128 + 8 + sgn) * (W + 16) + ch0,
                                      ap=[[W + 16, 128], [(RS + 16) * (W + 16), IN_C], [1, 528]])
                        if sgn == d:
                            nc.gpsimd.dma_start(S[:], src)
                        else:
                            nc.gpsimd.dma_start(S[:], src, accum_op=ALU.add)
                    # coefs
                    e1 = cpool.tile([128, 512], F32, tag="e1")
                    nc.scalar.activation(e1[:], gh[:, 4 + l, :], AF.Exp, scale=-1.0)
                    e2 = cpool.tile([128, 512], F32, tag="e2")
                    nc.scalar.activation(e2[:], e1[:], AF.Square)
                    den4 = cpool.tile([128, 512], F32, tag="den4", name="den4")
                    nc.vector.affine_then_add(den4[:], e1[:], e2[:], 1.0, 0.25)
                    rden = cpool.tile([128, 512], F32, tag="rden")
                    nc.vector.reciprocal_approx_fast(rden[:], den4[:])
                    u = cpool.tile([128, 512], F32, tag="u", name="u")
                    nc.vector.scalar_tensor_tensor(u[:], e_sm[:, l, :], 0.25, rsm[:],
                                                   op0=ALU.mult, op1=ALU.mult)
                    al = cpool.tile([128, 512], F32, tag="al")
                    nc.vector.tensor_mul(al[:], u[:], rden[:])
                    bl = cpool.tile([128, 512], F32, tag="bl")
                    nc.vector.tensor_mul(bl[:], al[:], e1[:])
                    gl = cpool.tile([128, 512], F32, tag="gl")
                    nc.vector.tensor_mul(gl[:], al[:], e2[:])
                    if first:
                        nc.vector.tensor_copy(asum[:], al[:])
                    else:
                        nc.vector.tensor_add(asum[:], asum[:], al[:])
                    # tap sums
                    A = fp1.tile([128, IN_C, 512], F32, tag="A", name="A")
                    nc.vector.tensor_add(A[:], v0[:, :, 8 - d:520 - d], v0[:, :, 8 + d:520 + d])
                    E = fp1.tile([128, IN_C, 512], F32, tag="E", name="E")
                    nc.vector.tensor_add(E[:], A[:], S[:, :, 8:520])
                    C = fp1.tile([128, IN_C, 512], F32, tag="C", name="C")
                    nc.vector.tensor_add(C[:], S[:, :, 8 - d:520 - d], S[:, :, 8 + d:520 + d])

                    def bcast(cf):
                        return bass.AP(tensor=cf.tensor, offset=cf.offset,
                                       ap=[cf.ap[0], [0, IN_C], cf.ap[1]])
                    tb = fp1.tile([128, IN_C, 512], F32, tag="tb", name="tb")
                    nc.vector.tensor_mul(tb[:], E[:], bcast(bl[:]))
                    if first:
                        nc.vector.tensor_mul(out[:], C[:], bcast(gl[:]))
                        first = False
                    else:
                        tg = fp1.tile([128, IN_C, 512], F32, tag="tg", name="tg")
                        nc.vector.tensor_mul(tg[:], C[:], bcast(gl[:]))
                        nc.vector.tensor_add(out[:], out[:], tg[:])
                    nc.vector.tensor_add(out[:], out[:], tb[:])
                # center term
                tcen = fp1.tile([128, IN_C, 512], F32, tag="tcen", name="tcen")
                nc.vector.tensor_mul(tcen[:], v0[:, :, 8:520], bcast_ap(asum[:], IN_C))
                nc.vector.tensor_add(out[:], out[:], tcen[:])
                # store
                dst = bass.AP(tensor=y_d.ap().tensor, offset=band * 128 * W + ch0,
                              ap=[[W, 128], [RS * W, IN_C], [1, 512]])
                nc.sync.dma_start(dst, out[:])

            def bcast_ap(cf, n):
                import concourse.bass as bass
                return bass.AP(tensor=cf.tensor, offset=cf.offset,
                               ap=[cf.ap[0], [0, n], cf.ap[1]])

            # ---- main pipeline ----
            for t in range(2):
                load_x_block(t)
            filt_done = 0
            for t in range(NBX + 3):
                load_x_block(t + 2)
                if t < NB[0]:
                    conv_block(0, t)
                if 1 <= t and t - 1 < NB[1]:
                    conv_block(1, t - 1)
                if 2 <= t and t - 2 < NB[2]:
                    conv_block(2, t - 2)
                if 3 <= t and t - 3 < NB[3]:
                    conv_block(3, t - 3)
                # guide block t-3 done; band b ready when blocks 16b..16b+15 done
                gdone = t - 2  # number of guide blocks completed
                while filt_done < 8 and gdone >= (filt_done // 2 + 1) * 16:
                    filtering(filt_done // 2, filt_done % 2)
                    filt_done += 1
            while filt_done < 8:
                filtering(filt_done // 2, filt_done % 2)
                filt_done += 1

    nc.compile()
    return nc


_CACHED = {}


def _get_compiled():
    if "nc" not in _CACHED:
        _install_ntff_hook()
        import concourse.bacc as bacc
        nc = bacc.Bacc("TRN2", target_bir_lowering=False, debug=False,
                       num_devices=N_CORES)
        _CACHED["nc"] = _build(nc)
    return _CACHED["nc"]


def kernel(imgs_in, params):
    imgs_in = np.asarray(imgs_in, np.float32)
    folded = _fold_params(params)
    wdict = _host_weights(folded)

    nc = _get_compiled()
    from concourse.bass_utils import run_bass_kernel_spmd

    in_maps = []
    for core in range(N_CORES):
        xconv, xfilt, masks, r0 = _host_inputs_for_core(core, imgs_in, folded)
        m = dict(xconv=xconv, xfilt=xfilt, masks=masks, biases=wdict["biases"],
                 zeros2=np.zeros((128, 2), np.float32))
        for k in ("w0g1", "w0g2", "w1g1", "w1g2", "w2g1", "w2g2", "w3g1", "w3g2"):
            m[k] = wdict[k]
        in_maps.append(m)

    res = run_bass_kernel_spmd(nc, in_maps, core_ids=list(range(N_CORES)),
                               trace=False)
    out = np.zeros((4, IN_C, H, W), np.float32)
    for core in range(N_CORES):
        b, half = core // 2, core % 2
        out[b, :, half * RS:(half + 1) * RS, :] = res.results[core]["y"]
    return out


# revision 8
# speedup vs baseline: 1.0792x; 1.0792x over previous
"""DenoiserNetwork Trainium2 kernel: 4-layer folded RepVGG conv stack + a-trous filtering.

Sharding: 8 cores = batch(4) x H-halves(2). Full inputs in, full output out.
"""
import sys, types
import numpy as np

IN_C, MID_C, NUM_LAYERS, KLEV = 3, 16, 4, 4
H = W = 1024
RS = 512          # rows per shard
N_CORES = 8
NBX = 67          # xconv blocks (rows r0-8-2pad.., 8 rows each)
NB = [66, 66, 65, 64]  # out-block counts for L0..L3
RING0 = 4
RING = 4


def _install_ntff_hook():
    try:
        import antenv
        if "antenv.axon_hooks" in sys.modules:
            return
        hooks_mod = types.ModuleType("antenv.axon_hooks")
        _h = [None]
        hooks_mod.set_axon_ntff_profile_hook = lambda h: _h.__setitem__(0, h)
        hooks_mod.get_axon_ntff_profile_hook = lambda: _h[0]
        sys.modules["antenv.axon_hooks"] = hooks_mod
        antenv.axon_hooks = hooks_mod
        from trn_agent_boot.trn_boot import _ntff_profile_via_ctypes
        hooks_mod.set_axon_ntff_profile_hook(
            _ntff_profile_via_ctypes('/opt/axon/libaxon_pjrt.so'))
    except Exception:
        pass


# ---------------------------------------------------------------- host prep --

def _fold_params(params):
    """RepVGG fold: each layer -> (Weff [co,ci,5,5], beff [co])."""
    folded = []
    for p in params:
        w5 = np.asarray(p['w5'], np.float32)
        w3 = np.asarray(p['w3'], np.float32)
        w1 = np.asarray(p['w1'], np.float32)
        co, ci = w5.shape[0], w5.shape[1]
        w = w5.copy()
        w[:, :, 1:4, 1:4] += w3
        w[:, :, 2:3, 2:3] += w1
        if co == ci:
            for c in range(co):
                w[c, c, 2, 2] += 1.0
        b = (np.asarray(p['b5']) + np.asarray(p['b3']) + np.asarray(p['b1'])).astype(np.float32)
        folded.append((w, b))
    return folded


def _pack_lhsT(weff, n_i, co_n, ci_n, packed_kw):
    """Build (g1, g2) lhsT arrays.

    g1[(j,(kw,)ci), (i*co_n+co)] = weff[co,ci,j-i,kw]   (0<=j-i<=4), j in 0..7
    g2[(j2,(kw,)ci), m]          = weff[co,ci,j2+8-i,kw], j2 in 0..3
    Without packed_kw returns per-kw lists.
    """
    co = co_n
    if packed_kw:
        g1 = np.zeros((8, 5, ci_n, n_i * co_n), np.float32)
        g2 = np.zeros((4, 5, ci_n, n_i * co_n), np.float32)
        for j in range(8):
            for kw in range(5):
                for ci in range(ci_n):
                    for i in range(n_i):
                        kh = j - i
                        if 0 <= kh <= 4:
                            g1[j, kw, ci, i * co_n:(i + 1) * co_n] = weff[:, ci, kh, kw]
                        kh2 = (j + 8 - i) if j < 4 else -1
                        if j < 4 and 0 <= kh2 <= 4:
                            g2[j, kw, ci, i * co_n:(i + 1) * co_n] = weff[:, ci, kh2, kw]
        return (g1.reshape(40, ci_n, n_i * co_n)[:, :, :].reshape(8 * 5 * ci_n, -1),
                g2[:4].reshape(4 * 5 * ci_n, -1))
    else:
        g1s, g2s = [], []
        for kw in range(5):
            g1 = np.zeros((8, ci_n, n_i * co_n), np.float32)
            g2 = np.zeros((4, ci_n, n_i * co_n), np.float32)
            for j in range(8):
                for ci in range(ci_n):
                    for i in range(n_i):
                        kh = j - i
                        if 0 <= kh <= 4:
                            g1[j, ci, i * co_n:(i + 1) * co_n] = weff[:, ci, kh, kw]
            for j2 in range(4):
                for ci in range(ci_n):
                    for i in range(n_i):
                        kh = j2 + 8 - i
                        if 0 <= kh <= 4:
                            g2[j2, ci, i * co_n:(i + 1) * co_n] = weff[:, ci, kh, kw]
            g1s.append(g1.reshape(8 * ci_n, -1))
            g2s.append(g2.reshape(4 * ci_n, -1))
        return np.stack(g1s, axis=1), np.stack(g2s, axis=1)


def _host_inputs_for_core(core, imgs, folded):
    b, half = core // 2, core % 2
    r0 = half * RS
    x = imgs[b]  # [3, H, W]

    # xconv: [120, NBX, 1024] kw-packed zero-padded slab, base row r0-10
    xc = np.zeros((8, 5, IN_C, NBX, W), np.float32)
    base0 = r0 - 8
    for j in range(8):
        rr = base0 + 8 * np.arange(NBX) + j  # [NBX]
        valid_r = (rr >= 0) & (rr < H)
        for kw in range(5):
            csrc = np.arange(W) + kw - 2
            valid_c = (csrc >= 0) & (csrc < W)
            cc = np.clip(csrc, 0, W - 1)
            block = x[:, np.clip(rr, 0, H - 1)[:, None], cc[None, :]]  # [3, NBX, W]
            block = block * valid_r[None, :, None] * valid_c[None, None, :]
            xc[j, kw] = block
    xconv = xc.reshape(120, NBX, W)

    # xfilt: [3, 528, 1040] circular wrap, rows r0-8.., cols -8..
    rr = (np.arange(RS + 16) + r0 - 8) % H
    cc = (np.arange(W + 16) - 8) % W
    xfilt = x[:, rr[:, None], cc[None, :]].astype(np.float32)

    # masks for slab1..3 out-blocks: mask_l[p, t] row-valid indicator
    masks = np.zeros((3, 128, 68), np.float32)
    for l, base in enumerate((r0 - 6, r0 - 4, r0 - 2)):
        for t in range(68):
            for j in range(8):
                r = base + 8 * t + j
                masks[l, j * 16:(j + 1) * 16, t] = 1.0 if 0 <= r < H else 0.0
    return xconv, xfilt, masks, r0


def _host_weights(folded):
    w0g1, w0g2 = _pack_lhsT(folded[0][0], 8, MID_C, IN_C, True)
    w1g1, w1g2 = _pack_lhsT(folded[1][0], 8, MID_C, MID_C, False)
    w2g1, w2g2 = _pack_lhsT(folded[2][0], 8, MID_C, MID_C, False)
    w3g1, w3g2 = _pack_lhsT(folded[3][0], 8, 2 * KLEV, MID_C, False)
    biases = np.zeros((4, 128), np.float32)
    for l in range(4):
        co_n = 2 * KLEV if l == 3 else MID_C
        for i in range(8):
            biases[l, i * co_n:(i + 1) * co_n] = folded[l][1]
    return dict(w0g1=w0g1, w0g2=w0g2, w1g1=w1g1, w1g2=w1g2,
                w2g1=w2g1, w2g2=w2g2, w3g1=w3g1, w3g2=w3g2, biases=biases)


# ---------------------------------------------------------------- bass build --

def _build(nc):
    import concourse.bass as bass
    import concourse.tile as tile
    from concourse import mybir
    dt, AF, ALU = mybir.dt, mybir.ActivationFunctionType, mybir.AluOpType
    F32, F32R = dt.float32, dt.float32r

    xconv_d = nc.dram_tensor("xconv", [120, NBX, W], F32R, kind="ExternalInput")
    xfilt_d = nc.dram_tensor("xfilt", [IN_C, RS + 16, W + 16], F32, kind="ExternalInput")
    masks_d = nc.dram_tensor("masks", [3, 128, 68], F32, kind="ExternalInput")
    w_d = {}
    wshapes = dict(w0g1=[120, 128], w0g2=[60, 128],
                   w1g1=[128, 5, 128], w1g2=[64, 5, 128],
                   w2g1=[128, 5, 128], w2g2=[64, 5, 128],
                   w3g1=[128, 5, 64], w3g2=[64, 5, 64])
    for k, shp in wshapes.items():
        w_d[k] = nc.dram_tensor(k, shp, F32R, kind="ExternalInput")
    bias_d = nc.dram_tensor("biases", [4, 128], F32, kind="ExternalInput")
    zeros_d = nc.dram_tensor("zeros2", [128, 2], F32R, kind="ExternalInput")
    y_d = nc.dram_tensor("y", [IN_C, RS, W], F32, kind="ExternalOutput")
    guide_dram = nc.dram_tensor("guide_scratch", [2 * KLEV, RS, W], F32)  # internal

    with tile.TileContext(nc) as tc:
        import contextlib
        ctx = contextlib.ExitStack()
        with ctx:
            singles = ctx.enter_context(tc.tile_pool(name="singles", bufs=1))
            psum = ctx.enter_context(tc.tile_pool(name="psum", bufs=8, space="PSUM"))
            stage = ctx.enter_context(tc.tile_pool(name="stage", bufs=2))
            fp2 = ctx.enter_context(tc.tile_pool(name="fp2", bufs=2))
            fp1 = ctx.enter_context(tc.tile_pool(name="fp1", bufs=1))
            cpool = ctx.enter_context(tc.tile_pool(name="cpool", bufs=1))

            # weights/bias/mask loads
            wt = {}
            for k, shp in wshapes.items():
                wt[k] = singles.tile(shp, F32R, name=f'w_{k}')
                nc.sync.dma_start(wt[k][:], w_d[k][:])
            bias_t = singles.tile([128, 4], F32)
            nc.sync.dma_start(bias_t[:], bias_d[:].rearrange("l p -> p l"))
            mask_t = singles.tile([128, 3, 68], F32)
            nc.sync.dma_start(mask_t[:], masks_d[:].rearrange("l p t -> p l t"))

            # ring slabs
            slab0 = singles.tile([120, RING0, W], F32R, name='slab0')
            slab = [singles.tile([128, RING, W + 4], F32R, name=f'slab{i+1}') for i in range(3)]
            # zero borders of slab1..3 (cols 0,1 and W+2,W+3) via broadcast DMA
            zsrc = bass.AP(tensor=zeros_d.ap().tensor, offset=0,
                           ap=[[2, 128], [0, RING], [1, 2]])
            for s in slab:
                nc.sync.dma_start(s[:, :, 0:2], zsrc)
                nc.sync.dma_start(s[:, :, W + 2:W + 4], zsrc)

            def load_x_block(t):
                if t < NBX:
                    nc.sync.dma_start(slab0[:, t % RING0, :], xconv_d[:, t, :])

            def conv_block(l, t):
                """Compute layer l out-block t (both halves)."""
                if l == 0:
                    src, g1, g2, co_n = slab0, wt["w0g1"], wt["w0g2"], MID_C
                    r_in = RING0
                else:
                    src = slab[l - 1]
                    g1, g2 = wt[f"w{l}g1"], wt[f"w{l}g2"]
                    co_n = 2 * KLEV if l == 3 else MID_C
                    r_in = RING
                m = 8 * co_n
                for h in range(2):
                    acc = psum.tile([128, 512], F32, tag="psacc")
                    if l == 0:
                        nc.tensor.matmul(acc[:m], g1[:], slab0[:, t % RING0, h * 512:h * 512 + 512],
                                         start=True, stop=False)
                        nc.tensor.matmul(acc[:m], g2[:], slab0[:60, (t + 1) % RING0, h * 512:h * 512 + 512],
                                         start=False, stop=True)
                    else:
                        for kw in range(5):
                            nc.tensor.matmul(acc[:m], g1[:, kw, :],
                                             src[:, t % r_in, h * 512 + kw:h * 512 + kw + 512],
                                             start=(kw == 0), stop=False)
                        for kw in range(5):
                            nc.tensor.matmul(acc[:m], g2[:, kw, :],
                                             src[:64, (t + 1) % r_in, h * 512 + kw:h * 512 + kw + 512],
                                             start=False, stop=(kw == 4))
                    # evacuate with relu + bias
                    if l < 3:
                        dst = slab[l][:m, t % RING, 2 + h * 512: 2 + h * 512 + 512]
                        if (t + h) % 2 == 0:
                            nc.scalar.activation(dst, acc[:m], AF.Relu,
                                                 bias=bias_t[:m, l:l + 1], scale=1.0)
                        else:
                            nc.vector.tensor_scalar(dst, acc[:m],
                                                    scalar1=bias_t[:m, l:l + 1], scalar2=0.0,
                                                    op0=ALU.add, op1=ALU.max)
                        if t <= 1 or t >= NB[l] - 2:
                            nc.vector.tensor_scalar_mul(dst, dst, mask_t[:m, l, t:t + 1])
                    else:
                        st = stage.tile([64, 512], F32, tag="gstage")
                        nc.scalar.activation(st[:], acc[:64], AF.Relu,
                                             bias=bias_t[:64, 3:4], scale=1.0)
                        # staging partitions m = i*8+co ; scatter to guide_dram[co, r0loc+8t+i, h*512+c]
                        dst_ap = bass.AP(
                            tensor=guide_dram.ap().tensor, offset=8 * t * W + h * 512,
                            ap=[[W, 8], [RS * W, 8], [1, 512]])
                        nc.sync.dma_start(dst_ap, st[:])

            # filtering constants
            DIL = [1, 2, 4, 8]

            def filtering(band, h):
                ch0 = h * 512
                gh = fp2.tile([128, 8, 512], F32, tag="guideh", name="gh")
                src_ap = bass.AP(tensor=guide_dram.ap().tensor,
                                 offset=band * 128 * W + ch0,
                                 ap=[[W, 128], [RS * W, 8], [1, 512]])
                nc.sync.dma_start(gh[:], src_ap)
                # softmax over ch 0..3 (no max-subtract; relu'd inputs are small)
                e_sm = fp1.tile([128, 4, 512], F32, tag="esm", name="e_sm")
                nc.scalar.activation(e_sm[:], gh[:, 0:4, :], AF.Exp)
                ssum = cpool.tile([128, 512], F32, tag="ssum")
                nc.vector.tensor_add(ssum[:], e_sm[:, 0, :], e_sm[:, 1, :])
                t2 = cpool.tile([128, 512], F32, tag="t2", name="t2")
                nc.vector.tensor_add(t2[:], e_sm[:, 2, :], e_sm[:, 3, :])
                nc.vector.tensor_add(ssum[:], ssum[:], t2[:])
                rsm = cpool.tile([128, 512], F32, tag="rsm")
                nc.vector.reciprocal_approx_fast(rsm[:], ssum[:])

                # v0 slab [128, 3, 528]
                v0 = fp1.tile([128, IN_C, 528], F32, tag="v0", name="v0")
                src = bass.AP(tensor=xfilt_d.ap().tensor,
                              offset=(band * 128 + 8) * (W + 16) + ch0,
                              ap=[[W + 16, 128], [(RS + 16) * (W + 16), IN_C], [1, 528]])
                nc.sync.dma_start(v0[:], src)

                out = fp1.tile([128, IN_C, 512], F32, tag="outb", name="out")
                asum = cpool.tile([128, 512], F32, tag="asum")
                first = True
                for l in range(KLEV):
                    d = DIL[l]
                    # S = xfilt[+d] + xfilt[-d] via accumulating DMA
                    S = fp1.tile([128, IN_C, 528], F32, tag="S", name="S")
                    for sgn in (d, -d):
                        src = bass.AP(tensor=xfilt_d.ap().tensor,
                                      offset=(band * 128 + 8 + sgn) * (W + 16) + ch0,
                                      ap=[[W + 16, 128], [(RS + 16) * (W + 16), IN_C], [1, 528]])
                        if sgn == d:
                            nc.gpsimd.dma_start(S[:], src)
                        else:
                            nc.gpsimd.dma_start(S[:], src, accum_op=ALU.add)
                    # coefs
                    e1 = cpool.tile([128, 512], F32, tag="e1")
                    nc.scalar.activation(e1[:], gh[:, 4 + l, :], AF.Exp, scale=-1.0)
                    e2 = cpool.tile([128, 512], F32, tag="e2")
                    nc.scalar.activation(e2[:], e1[:], AF.Square)
                    den4 = cpool.tile([128, 512], F32, tag="den4", name="den4")
                    nc.vector.affine_then_add(den4[:], e1[:], e2[:], 1.0, 0.25)
                    rden = cpool.tile([128, 512], F32, tag="rden")
                    nc.vector.reciprocal_approx_fast(rden[:], den4[:])
                    u = cpool.tile([128, 512], F32, tag="u", name="u")
                    nc.vector.scalar_tensor_tensor(u[:], e_sm[:, l, :], 0.25, rsm[:],
                                                   op0=ALU.mult, op1=ALU.mult)
                    al = cpool.tile([128, 512], F32, tag="al")
                    nc.vector.tensor_mul(al[:], u[:], rden[:])
                    bl = cpool.tile([128, 512], F32, tag="bl")
                    nc.vector.tensor_mul(bl[:], al[:], e1[:])
                    gl = cpool.tile([128, 512], F32, tag="gl")
                    nc.vector.tensor_mul(gl[:], al[:], e2[:])
                    if first:
                        nc.vector.tensor_copy(asum[:], al[:])
                    else:
                        nc.vector.tensor_add(asum[:], asum[:], al[:])
                    # tap sums
                    A = fp1.tile([128, IN_C, 512], F32, tag="A", name="A")
                    nc.gpsimd.tensor_add(A[:], v0[:, :, 8 - d:520 - d], v0[:, :, 8 + d:520 + d])
                    E = fp1.tile([128, IN_C, 512], F32, tag="E", name="E")
                    nc.vector.tensor_add(E[:], A[:], S[:, :, 8:520])
                    C = fp1.tile([128, IN_C, 512], F32, tag="C", name="C")
                    nc.gpsimd.tensor_add(C[:], S[:, :, 8 - d:520 - d], S[:, :, 8 + d:520 + d])

                    def bcast(cf):
                        return bass.AP(tensor=cf.tensor, offset=cf.offset,
                                       ap=[cf.ap[0], [0, IN_C], cf.ap[1]])
                    tb = fp1.tile([128, IN_C, 512], F32, tag="tb", name="tb")
                    nc.vector.tensor_mul(tb[:], E[:], bcast(bl[:]))
                    if first:
                        nc.vector.tensor_mul(out[:], C[:], bcast(gl[:]))
                        first = False
                    else:
                        tg = fp1.tile([128, IN_C, 512], F32, tag="tg", name="tg")
                        nc.vector.tensor_mul(tg[:], C[:], bcast(gl[:]))
                        nc.vector.tensor_add(out[:], out[:], tg[:])
                    nc.vector.tensor_add(out[:], out[:], tb[:])
                # center term
                tcen = fp1.tile([128, IN_C, 512], F32, tag="tcen", name="tcen")
                nc.vector.tensor_mul(tcen[:], v0[:, :, 8:520], bcast_ap(asum[:], IN_C))
                nc.vector.tensor_add(out[:], out[:], tcen[:])
                # store
                dst = bass.AP(tensor=y_d.ap().tensor, offset=band * 128 * W + ch0,
                              ap=[[W, 128], [RS * W, IN_C], [1, 512]])
                nc.sync.dma_start(dst, out[:])

            def bcast_ap(cf, n):
                import concourse.bass as bass
                return bass.AP(tensor=cf.tensor, offset=cf.offset,
                               ap=[cf.ap[0], [0, n], cf.ap[1]])

            # ---- main pipeline ----
            for t in range(2):
                load_x_block(t)
            filt_done = 0
            for t in range(NBX + 3):
                load_x_block(t + 2)
                if t < NB[0]:
                    conv_block(0, t)
                if 1 <= t and t - 1 < NB[1]:
                    conv_block(1, t - 1)
                if 2 <= t and t - 2 < NB[2]:
                    conv_block(2, t - 2)
                if 3 <= t and t - 3 < NB[3]:
                    conv_block(3, t - 3)
                # guide block t-3 done; band b ready when blocks 16b..16b+15 done
                gdone = t - 2  # number of guide blocks completed
                while filt_done < 8 and gdone >= (filt_done // 2 + 1) * 16:
                    filtering(filt_done // 2, filt_done % 2)
                    filt_done += 1
            while filt_done < 8:
                filtering(filt_done // 2, filt_done % 2)
                filt_done += 1

    nc.compile()
    return nc


_CACHED = {}


def _get_compiled():
    if "nc" not in _CACHED:
        _install_ntff_hook()
        import concourse.bacc as bacc
        nc = bacc.Bacc("TRN2", target_bir_lowering=False, debug=False,
                       num_devices=N_CORES)
        _CACHED["nc"] = _build(nc)
    return _CACHED["nc"]


def kernel(imgs_in, params):
    imgs_in = np.asarray(imgs_in, np.float32)
    folded = _fold_params(params)
    wdict = _host_weights(folded)

    nc = _get_compiled()
    from concourse.bass_utils import run_bass_kernel_spmd

    in_maps = []
    for core in range(N_CORES):
        xconv, xfilt, masks, r0 = _host_inputs_for_core(core, imgs_in, folded)
        m = dict(xconv=xconv, xfilt=xfilt, masks=masks, biases=wdict["biases"],
                 zeros2=np.zeros((128, 2), np.float32))
        for k in ("w0g1", "w0g2", "w1g1", "w1g2", "w2g1", "w2g2", "w3g1", "w3g2"):
            m[k] = wdict[k]
        in_maps.append(m)

    res = run_bass_kernel_spmd(nc, in_maps, core_ids=list(range(N_CORES)),
                               trace=False)
    out = np.zeros((4, IN_C, H, W), np.float32)
    for core in range(N_CORES):
        b, half = core // 2, core % 2
        out[b, :, half * RS:(half + 1) * RS, :] = res.results[core]["y"]
    return out
